# revision 11
# baseline (speedup 1.0000x reference)
"""Expert-parallel MoE MLP kernel for TRN2 (8 NeuronCores).

Reference computation (all experts, dense routing):
    hidden = einsum("bnd,edh->benh", x, w1); hidden = gelu(hidden)
    out    = einsum("benh,ehd->bnde", hidden, w2)        # [b, n, d4, e]

Sharding: expert-parallel, 2 experts per core (16 experts / 8 cores); x is
replicated. Each core computes, for its experts e:
    hT[e] = gelu(W1[e].T @ X.T)        # [h, tok] layout, h on partitions
    outT[e] = W2[e].T @ hT[e]          # [d4, tok] layout
which keeps the contraction dim on SBUF partitions for both matmuls with no
on-device transposes: W1 (d,h) / W2 (h,d4) load in natural layout as lhsT, and
X.T is prepared once on the host.

All operands are bf16 (PSUM accumulation stays fp32): same PE throughput as
fp32r (1 row/cycle at N=512) but the compiler's automatic fast-weight-load
engages for 16-bit weights, hiding LDWEIGHTS under the previous matmul's
streaming, and DMA traffic halves (x 8MB, w 1.25MB, out 4MB per core).
The [e, d4, tok] bf16 device layout is cast and re-interleaved to the
[b, n, d4, e] fp32 output on the host.
"""

import sys

import numpy as np

for _p in ("/opt/trn_rl_repo", "/root/.axon_site/_ro/trn_rl_repo"):
    if _p not in sys.path:
        sys.path.append(_p)

import ml_dtypes

import concourse.bacc as bacc
import concourse.mybir as mybir
import concourse.tile as tile
from concourse.bass_utils import run_bass_kernel_spmd

F32 = mybir.dt.float32
BF16 = mybir.dt.bfloat16
NP_BF16 = ml_dtypes.bfloat16

N_CORES = 8
E = 16                 # total experts
E_LOC = E // N_CORES   # experts per core
D = 512                # model dim (contraction of mm1)
H = 512                # hidden dim (contraction of mm2)
D4 = 128               # output dim per expert
NTOK = 4 * 2048        # tokens
TT = 512               # token tile (matmul moving free dim)
P = 128


def _build_program():
    nc = bacc.Bacc("TRN2", target_bir_lowering=False, debug=False)
    xT = nc.declare_dram_parameter("xT", [D, NTOK], BF16, isOutput=False)
    w1 = nc.declare_dram_parameter("w1", [E_LOC, D, H], BF16, isOutput=False)
    w2 = nc.declare_dram_parameter("w2", [E_LOC, H, D4], BF16, isOutput=False)
    outT = nc.declare_dram_parameter("outT", [E_LOC, D4, NTOK], BF16, isOutput=True)

    gelu = mybir.ActivationFunctionType.Gelu
    n_dt = D // P   # 4 k-tiles of mm1
    n_ht = H // P   # 4 k-tiles of mm2

    n_t = NTOK // TT

    with tile.TileContext(nc) as tc:
        with (
            tc.tile_pool(name="wpool", bufs=1) as wpool,
            tc.tile_pool(name="xpool", bufs=4) as xpool,
            tc.tile_pool(name="hpool", bufs=2) as hpool,
            tc.tile_pool(name="opool", bufs=4) as opool,
            tc.tile_pool(name="ps1p", bufs=4, space="PSUM") as ps1p,
            tc.tile_pool(name="ps2p", bufs=4, space="PSUM") as ps2p,
        ):
            # Weights resident in SBUF for the whole kernel, natural layout.
            w1_sb = wpool.tile([P, E_LOC, n_dt, H], BF16, name="w1_sb", tag="w1")
            w1_r = w1.rearrange("e (dt p) h -> p e dt h", p=P)
            w2_sb = wpool.tile([P, E_LOC, n_ht, D4], BF16, name="w2_sb", tag="w2")
            w2_r = w2.rearrange("e (ht p) d -> p e ht d", p=P)
            xT_r = xT.rearrange("(dt p) n -> p dt n", p=P)

            # PE warmup: dummy matmuls with no DMA dependency keep the PE busy
            # through the initial x0/w1 transfer window so the p-state is fully
            # ramped (2.4 GHz) when the first real chain starts. Sized to end
            # right as the startup DMAs complete (~11.5us). The warmup PSUM
            # reuses a ps2 pool slot before its first real use.
            junk = wpool.tile([P, TT], BF16, name="junk", tag="junk")
            nc.vector.memset(junk, 0.0)
            wu_ps = ps2p.tile([P, TT], F32, name="wu_ps", tag="ps2")
            for _ in range(10):
                nc.tensor.matmul(wu_ps, junk[:, :P], junk, start=True, stop=True)

            outT_r = outT.rearrange("e d n -> d e n")

            x_tiles = {}

            def load_x(t, n_tiles=1):
                tok = slice(t * TT, (t + n_tiles) * TT)
                x_sb = xpool.tile([P, n_dt, n_tiles * TT], BF16, name="x_sb", tag="x")
                nc.sync.dma_start(x_sb, xT_r[:, :, tok])
                for k in range(n_tiles):
                    x_tiles[t + k] = (x_sb, k * TT)

            # Startup DMAs: x0 first, then w1[e0] in ht-column blocks so chain
            # (e0, ht0) only needs x0 + 128KB of w1; w1[e1] lands during
            # mm1(e0); w2 after.
            tok0 = slice(0, TT)
            x0_sb = xpool.tile([P, n_dt, TT], BF16, name="x_sb", tag="x")
            nc.sync.dma_start(x0_sb, xT_r[:, :, tok0])
            for ht in range(n_ht):
                nc.sync.dma_start(
                    w1_sb[:, 0, :, ht * P : (ht + 1) * P],
                    w1_r[:, 0, :, ht * P : (ht + 1) * P],
                )
            x_tiles[0] = (x0_sb, 0)
            for e in range(1, E_LOC):
                nc.sync.dma_start(w1_sb[:, e], w1_r[:, e])
            nc.sync.dma_start(w2_sb, w2_r)

            def mm1(e, x_ref):
                """One expert's mm1 + gelu for a token tile -> hT tile."""
                x_sb, off = x_ref
                hT_sb = hpool.tile([P, n_ht, TT], BF16, name="hT_sb", tag="h")
                for ht in range(n_ht):
                    ps1 = ps1p.tile([P, TT], F32, name="ps1", tag="ps1")
                    for dt_i in range(n_dt):
                        nc.tensor.matmul(
                            ps1,
                            w1_sb[:, e, dt_i, ht * P : (ht + 1) * P],
                            x_sb[:, dt_i, off : off + TT],
                            start=(dt_i == 0),
                            stop=(dt_i == n_dt - 1),
                        )
                    nc.scalar.activation(hT_sb[:, ht, :], ps1, gelu)
                return hT_sb

            def mm2_final(e, hT_sb, tok, n_split):
                ntt = TT // n_split
                for s in range(n_split):
                    ts_ = slice(s * ntt, (s + 1) * ntt)
                    ps2 = ps2p.tile([P, ntt], F32, name="ps2", tag="ps2")
                    for ht in range(n_ht):
                        nc.tensor.matmul(
                            ps2,
                            w2_sb[:, e, ht, :],
                            hT_sb[:, ht, ts_],
                            start=(ht == 0),
                            stop=(ht == n_ht - 1),
                        )
                    o_sb = opool.tile([P, ntt], BF16, name="o_sb", tag="o")
                    nc.vector.tensor_copy(o_sb, ps2)
                    nc.sync.dma_start(
                        outT[e, :, tok.start + s * ntt : tok.start + (s + 1) * ntt],
                        o_sb,
                    )

            # x prefetch: tile1 alone (needed early), then pairs with >=2 tiles
            # of lead; one DMA (and one cleanup semaphore) per pair.
            prefetch = {-1: [(1, 1)]}
            k = 2
            while k < n_t:
                n_pair = min(2, n_t - k)
                prefetch.setdefault(k - 2, []).append((k, n_pair))
                k += n_pair

            # Software-pipelined schedule: each tile's mm2 chains run AFTER the
            # next tile's mm1 has been interleaved, so mm2 never waits on the
            # gelu that produced its hT input.
            # PE order: mm1(0,e0) mm1(0,e1) | mm2(0,e0) mm1(1,e0) mm2(0,e1)
            # mm1(1,e1) | mm2(1,e0) mm1(2,e0) ...
            for t, np_ in prefetch[-1]:
                load_x(t, np_)
            x0_ref = x_tiles.pop(0)
            hT_cur = [mm1(e, x0_ref) for e in range(E_LOC)]
            for t in range(n_t):
                tok = slice(t * TT, (t + 1) * TT)
                for tk, np_ in prefetch.get(t, []):
                    load_x(tk, np_)
                nxt = t + 1
                x_nxt = x_tiles.pop(nxt) if nxt < n_t else None
                hT_nxt = [None] * E_LOC
                if nxt < n_t:
                    # interleave: mm2(t,e0); mm1(t+1,e0); mm2(t,e1); mm1(t+1,e1)
                    # but keep the merged out DMA after both casts
                    o_sb = opool.tile([P, E_LOC, TT], BF16, name="o_sb", tag="o")
                    for e in range(E_LOC):
                        ps2 = ps2p.tile([P, TT], F32, name="ps2", tag="ps2")
                        for ht in range(n_ht):
                            nc.tensor.matmul(
                                ps2,
                                w2_sb[:, e, ht, :],
                                hT_cur[e][:, ht, :],
                                start=(ht == 0),
                                stop=(ht == n_ht - 1),
                            )
                        nc.vector.tensor_copy(o_sb[:, e], ps2)
                        hT_nxt[e] = mm1(e, x_nxt)
                    nc.sync.dma_start(outT_r[:, :, tok], o_sb)
                else:
                    # final tile: separate small outputs so the last DMA is tiny
                    mm2_final(0, hT_cur[0], tok, n_split=1)
                    mm2_final(1, hT_cur[1], tok, n_split=2)
                hT_cur = hT_nxt

    nc.finalize()
    return nc


_NC = None


def _get_program():
    global _NC
    if _NC is None:
        _NC = _build_program()
    return _NC


def _prep_inputs(x, w1, w2):
    xT = np.ascontiguousarray(x.reshape(NTOK, D).T).astype(NP_BF16)
    w1b = w1.astype(NP_BF16)
    w2b = w2.astype(NP_BF16)
    return [
        {
            "xT": xT,
            "w1": np.ascontiguousarray(w1b[c * E_LOC : (c + 1) * E_LOC]),
            "w2": np.ascontiguousarray(w2b[c * E_LOC : (c + 1) * E_LOC]),
        }
        for c in range(N_CORES)
    ]


def kernel(x: np.ndarray, w1: np.ndarray, w2: np.ndarray, **_) -> np.ndarray:
    """Full inputs in, full output out; expert-parallel across 8 NeuronCores."""
    nc = _get_program()
    in_maps = _prep_inputs(x, w1, w2)
    res = run_bass_kernel_spmd(nc, in_maps, list(range(N_CORES)))

    full = np.stack(
        [np.asarray(res.results[c]["outT"]) for c in range(N_CORES)], axis=0
    ).astype(np.float32)
    full = full.reshape(E, D4, NTOK)              # [e, d4, tok]
    out = full.transpose(2, 1, 0)                 # [tok, d4, e]
    return np.ascontiguousarray(out.reshape(4, 2048, D4, E), dtype=np.float32)
